# revision 42
# baseline (speedup 1.0000x reference)
"""GCN (3x GCNConv + global max pool + MLP) on 8 Trainium2 NeuronCores.

Strategy (graph/data parallel, per sharding hint):
  - 2048 graphs are split 256-per-core; each core owns the (contiguous) nodes
    of its graphs. Edges are assigned to the core owning their *dst* node, so
    all segment reductions are core-local.
  - GCN algebra is reordered as out = dinv*(A @ (dinv*x)) @ W + b (aggregate
    before transform), which gathers F_in-wide rows instead of F_out-wide.
  - Gathers use dma_gather (int16 indices => the global node table is split in
    two halves; per dst-window edges are grouped by half, two passes A/B).
  - Scatter-add is a TensorE matmul with a one-hot "S" matrix built on DVE:
    S[e, m] = (iota[m] == dst_rel[e]) * dinv_dst[e]; aggT = msg.T @ S.
  - Between conv layers the (dinv-prescaled) node features are AllGather'd so
    every core can gather any source row.
  - Global max pool: per-graph fixed 64-slot gather of h3 rows + TensorE
    transpose + DVE reduce_max; MLP runs feature-major with per-partition
    biases; sigmoid on ACT.
"""
import os
import sys

sys.path.insert(0, "/opt/trn_rl_repo")

import numpy as np

import concourse.bass as bass
import concourse.mybir as mybir
import concourse.tile as tile
from concourse import bacc
from concourse.bass_utils import run_bass_kernel_spmd
from concourse.masks import make_identity

F32 = mybir.dt.float32
F16 = mybir.dt.float16
I16 = mybir.dt.int16
NC = 8
WIN = 64  # dst-window size (nodes per scatter matmul group)
NPF16 = np.float16


def _ceil(a, b):
    return -(-a // b)


def _pad_elem(f):
    # gather row length (fp16 elems): must make row bytes a multiple of 256
    return _ceil(f, 128) * 128


def _ksplits(k):
    # split contraction dim k into <=128 chunks
    out = []
    o = 0
    while o < k:
        out.append((o, min(o + 128, k)))
        o += 128
    return out


def _wrap_idx(flat):
    # dma_gather index layout: idx i -> partition i%16, col i//16, replicated x8
    w = flat.reshape(-1, 16).T.astype(np.int16)
    return np.tile(w, (8, 1))


def _plan(x, edge_index, batch, weights):
    N, XD = x.shape
    E = edge_index.shape[1]
    G = 2048 if N == 50000 else int(batch.max()) + 1
    assert G % NC == 0
    GPC = G // NC

    batch = np.asarray(batch, dtype=np.int64)
    sizes = np.bincount(batch, minlength=G)
    gcore = np.arange(G) // GPC
    node_core = gcore[batch]

    core_start = np.searchsorted(batch, np.arange(NC) * GPC)
    core_start = np.concatenate([core_start, [N]])
    ncounts = np.diff(core_start)
    N_PAD = max(128, _ceil(ncounts.max(), 128) * 128)
    assert NC * N_PAD < 65536, "node table too large for int16 half-split"
    HALF = NC * N_PAD // 2
    NW = N_PAD // WIN

    local_row = np.arange(N) - core_start[node_core]
    NCH_CC = 1  # single full-table AllGather (best modeled bandwidth)
    CR = N_PAD // NCH_CC
    NSEG = 2  # window segments (decoupled from collectives; pooling overlap)
    SEGR = N_PAD // NSEG
    assert N_PAD % NCH_CC == 0 and CR % WIN == 0 and SEGR % WIN == 0
    g_row = ((local_row // CR) * (NC * CR) + node_core * CR + local_row % CR)

    src = np.asarray(edge_index[0], dtype=np.int64)
    dst = np.asarray(edge_index[1], dtype=np.int64)
    deg = (np.bincount(dst, minlength=N) + 1).astype(np.float32)
    dinv = (1.0 / np.sqrt(deg)).astype(np.float32)

    src_all = np.concatenate([src, np.arange(N)])
    dst_all = np.concatenate([dst, np.arange(N)])
    e_core = node_core[dst_all]
    e_loc = local_row[dst_all]
    e_w = e_loc // WIN
    e_rel = (e_loc % WIN).astype(np.float32)
    e_grow = g_row[src_all]
    e_half = (e_grow >= HALF).astype(np.int64)
    e_idx16 = (e_grow - e_half * HALF).astype(np.int16)
    e_sval = dinv[dst_all]

    key = (e_core * 2 + e_half) * NW + e_w
    cnt = np.bincount(key, minlength=NC * 2 * NW).reshape(NC, 2, NW)
    K = np.maximum(1, _ceil(cnt.max(axis=0), 128))  # [2, NW] blocks per (half, w)
    NBLK_A, NBLK_B = int(K[0].sum()), int(K[1].sum())
    NBLK = NBLK_A + NBLK_B
    E_cap = NBLK * 128

    # block start offset of each (half, w) group; stream is ordered
    # [seg0: h0 w0..w_cpw-1, h1 w0..][seg1: h0 ...] so each collective
    # chunk's windows finish early in the layer
    CPW = SEGR // WIN
    blk_off = np.zeros((2, NW), dtype=np.int64)
    off = 0
    for s in range(NSEG):
        for h in range(2):
            for w in range(s * CPW, (s + 1) * CPW):
                blk_off[h, w] = off
                off += int(K[h, w])
    assert off == NBLK

    order = np.lexsort((e_w, e_half, e_core))
    s_key = key[order]
    s_idx16 = e_idx16[order]
    s_rel = e_rel[order]
    s_sval = e_sval[order]
    grp_starts = np.searchsorted(s_key, np.arange(NC * 2 * NW))
    grp_ends = np.concatenate([grp_starts[1:], [E + N]])

    idx_w, rel_cols, sval_cols = [], [], []
    for c in range(NC):
        idx16_s = np.zeros(E_cap, np.int16)
        rel_s = np.full(E_cap, -1.0, np.float32)
        sval_s = np.zeros(E_cap, np.float32)
        for h in range(2):
            for w in range(NW):
                kk = (c * 2 + h) * NW + w
                a, b = grp_starts[kk], grp_ends[kk]
                m = b - a
                if m == 0:
                    continue
                d0 = blk_off[h, w] * 128
                idx16_s[d0 : d0 + m] = s_idx16[a:b]
                rel_s[d0 : d0 + m] = s_rel[a:b]
                sval_s[d0 : d0 + m] = s_sval[a:b]
        idx_w.append(_wrap_idx(idx16_s))
        rel_cols.append(np.ascontiguousarray(rel_s.reshape(NBLK, 128).T))
        sval_cols.append(np.ascontiguousarray(sval_s.reshape(NBLK, 128).T))

    # ---- push-mode stream for layers 2/3 (shared): edges grouped by
    # (dst core, 128-wide dst window); messages gathered from the LOCAL
    # node table (src side), scatter-matmul'd into a feature-major global
    # accumulator, then ReduceScatter sums partials onto the dst core.
    WIN2 = 256
    NW2 = N_PAD // WIN2
    # self-loops are NOT streamed: each core adds its own dinv^2-scaled
    # rows to its shard post-ReduceScatter (keeps the max-over-cores block
    # padding flat: the diagonal (own-core) groups would otherwise carry
    # all self-loops and inflate every window's K)
    p_src_core = node_core[src]
    p_loc_src = local_row[src]
    p_dst_core = node_core[dst]
    p_loc_dst = local_row[dst]
    p_w = p_loc_dst // WIN2
    p_rel = (p_loc_dst % WIN2).astype(np.float32)
    p_sval = dinv[dst]
    key2 = (p_src_core * NC + p_dst_core) * NW2 + p_w
    cnt2 = np.bincount(key2, minlength=NC * NC * NW2).reshape(NC, NC, NW2)
    K2 = np.maximum(1, _ceil(cnt2.max(axis=0), 128))  # [NC, NW2]
    NBLK2 = int(K2.sum())
    E2_cap = NBLK2 * 128
    blk_off2 = np.concatenate([[0], np.cumsum(K2.reshape(-1))])[:-1].reshape(
        NC, NW2)

    order2 = np.argsort(key2, kind="stable")
    s2_key = key2[order2]
    s2_idx = p_loc_src[order2].astype(np.int16)
    s2_rel = p_rel[order2]
    s2_sval = p_sval[order2]
    idx2_w, rel2_cols, sval2_cols = [], [], []
    for e in range(NC):
        g0 = np.searchsorted(s2_key, (e * NC) * NW2)
        g1 = np.searchsorted(s2_key, ((e + 1) * NC) * NW2)
        idx16_s = np.zeros(E2_cap, np.int16)
        rel_s = np.full(E2_cap, -1.0, np.float32)
        sval_s = np.zeros(E2_cap, np.float32)
        kk = s2_key[g0:g1] - (e * NC) * NW2  # (c, w) flat group id
        grp_starts2 = np.searchsorted(kk, np.arange(NC * NW2))
        grp_ends2 = np.concatenate([grp_starts2[1:], [g1 - g0]])
        flat_off = blk_off2.reshape(-1)
        for g in range(NC * NW2):
            a, b = grp_starts2[g], grp_ends2[g]
            if a == b:
                continue
            d0 = int(flat_off[g]) * 128
            idx16_s[d0 : d0 + b - a] = s2_idx[g0 + a : g0 + b]
            rel_s[d0 : d0 + b - a] = s2_rel[g0 + a : g0 + b]
            sval_s[d0 : d0 + b - a] = s2_sval[g0 + a : g0 + b]
        idx2_w.append(_wrap_idx(idx16_s))
        rel2_cols.append(np.ascontiguousarray(rel_s.reshape(NBLK2, 128).T))
        sval2_cols.append(np.ascontiguousarray(sval_s.reshape(NBLK2, 128).T))

    # pooling slot plan
    maxg = int(sizes.max())
    SLOT = 64 if maxg <= 64 else 128
    assert maxg <= SLOT, f"graph with {maxg} nodes exceeds slot budget"
    NSLOT = GPC * SLOT
    assert NSLOT % 128 == 0
    # h3 table row 0 is an all-zero dummy; node local row r lives at row r+1
    gstart = np.concatenate([[0], np.cumsum(sizes)])
    slot_w = []
    jstar = GPC
    for c in range(NC):
        sl = np.zeros(NSLOT, np.int64)
        for j in range(GPC):
            gi = c * GPC + j
            st = gstart[gi] - core_start[c]
            sz = sizes[gi]
            sl[j * SLOT : j * SLOT + sz] = np.arange(st + 1, st + sz + 1)
        slot_w.append(_wrap_idx(sl.astype(np.int16)))
        ge = gstart[c * GPC + 1 : (c + 1) * GPC + 1] - core_start[c]
        jstar = min(jstar, int(np.searchsorted(ge, SEGR, side="right")))
    jstar -= jstar % max(1, 128 // SLOT)  # slot-tile granularity
    # graphs complete per 128-row transform chunk (min over cores), in
    # slot-tile units: lets pooling chase the L3 transform chunk-by-chunk
    jchunk = np.full(N_PAD // 128, GPC, np.int64)
    for c in range(NC):
        ge = gstart[c * GPC + 1 : (c + 1) * GPC + 1] - core_start[c]
        for ch in range(N_PAD // 128):
            j = int(np.searchsorted(ge, (ch + 1) * 128, side="right"))
            jchunk[ch] = min(jchunk[ch], j)
    gpt = max(1, 128 // SLOT)  # graphs per slot-tile
    jchunk -= jchunk % gpt

    # per-core node-local aux data; T1 (= dinv*x, padded, chunk-major
    # global layout) is built on the host and uploaded replicated
    dinv_win = []
    dinv_c128 = []
    dinvsq_win = []
    dinvsq_c128 = []
    E1 = _pad_elem(XD)
    T1_host = np.zeros((NC * N_PAD, E1), NPF16)
    xs = (dinv[:, None] * np.asarray(x, np.float32)).astype(NPF16)
    T1_host[g_row, :XD] = xs
    for c in range(NC):
        n0, n1 = core_start[c], core_start[c + 1]
        dl = np.zeros(N_PAD, np.float32)
        dl[: n1 - n0] = dinv[n0:n1]
        dinv_win.append(np.ascontiguousarray(dl.reshape(NW, WIN).T))
        dinv_c128.append(np.ascontiguousarray(dl.reshape(-1, 128).T))
        dinvsq_win.append(np.ascontiguousarray((dl * dl).reshape(NW, WIN).T))
        dinvsq_c128.append(np.ascontiguousarray((dl * dl).reshape(-1, 128).T))

    W1, b1, W2, b2, W3, b3 = (
        weights["W1"], weights["b1"], weights["W2"], weights["b2"],
        weights["W3"], weights["b3"],
    )
    F1, F2, F3 = W1.shape[1], W2.shape[1], W3.shape[1]
    w_shared = dict(
        W1e=np.vstack([W1, b1[None, :]]).astype(NPF16),
        W2e=np.vstack([W2, b2[None, :]]).astype(NPF16),
        W3lo=np.ascontiguousarray(np.asarray(W3[:128], NPF16)),
        W3hi=np.vstack([W3[128:], b3[None, :]]).astype(NPF16),
        Wg1=np.asarray(weights["Wg1"], NPF16),
        Wg2=np.asarray(weights["Wg2"], NPF16),
        Wf1=np.asarray(weights["Wf1"], NPF16),
        Wf2=np.asarray(weights["Wf2"], NPF16),
        Wo=np.asarray(weights["Wo"], NPF16),
        bg1=np.ascontiguousarray(
            np.asarray(weights["bg1"], np.float32).reshape(-1, 128).T),
        bg2=np.asarray(weights["bg2"], np.float32).reshape(128, -1),
        bf1=np.ascontiguousarray(
            np.asarray(weights["bf1"], np.float32).reshape(-1, 128).T),
        bf2=np.ascontiguousarray(
            np.asarray(weights["bf2"], np.float32).reshape(-1, 128).T),
        bo=np.asarray(weights["bo"], np.float32).reshape(1, 1),
        iota=np.broadcast_to(
            np.arange(WIN2, dtype=NPF16)[None, :], (128, WIN2)).copy(),
    )

    meta = dict(
        N=N, XD=XD, G=G, GPC=GPC, N_PAD=N_PAD, HALF=HALF, NW=NW,
        K=K, NBLK_A=NBLK_A, NBLK=NBLK, E_cap=E_cap, NCH_CC=NCH_CC, CR=CR,
        CPW=CPW, NSEG=NSEG, SEGR=SEGR,
        WIN2=WIN2, NW2=NW2, K2=K2, NBLK2=NBLK2, E2_cap=E2_cap,
        blk_off2=blk_off2,
        SLOT=SLOT, NSLOT=NSLOT, JSTAR=jstar, JCHUNK=jchunk,
        F1=F1, F2=F2, F3=F3,
        D1=weights["Wg1"].shape[1], D2=weights["Wg2"].shape[1],
        D3=weights["Wf1"].shape[1], D4=weights["Wf2"].shape[1],
    )
    in_maps = [
        dict(
            T1_in=T1_host, idx_w=idx_w[c], rel_c=rel_cols[c],
            sval_c=sval_cols[c], slot_w=slot_w[c],
            idx2_w=idx2_w[c], rel2_c=rel2_cols[c], sval2_c=sval2_cols[c],
            dinv_win=dinv_win[c], dinv_c=dinv_c128[c],
            dinvsq_win=dinvsq_win[c], dinvsq_c=dinvsq_c128[c],
            **w_shared,
        )
        for c in range(NC)
    ]
    return meta, in_maps


def _build(meta):
    N_PAD, HALF, NW = meta["N_PAD"], meta["HALF"], meta["NW"]
    NCH_CC, CR = meta["NCH_CC"], meta["CR"]
    K, NBLK, E_cap = meta["K"], meta["NBLK"], meta["E_cap"]
    GPC, SLOT, NSLOT = meta["GPC"], meta["SLOT"], meta["NSLOT"]
    F1, F2, F3, XD = meta["F1"], meta["F2"], meta["F3"], meta["XD"]
    D1, D2, D3, D4 = meta["D1"], meta["D2"], meta["D3"], meta["D4"]
    E1, E2, E3 = _pad_elem(XD), _pad_elem(F1), _pad_elem(F2)
    EP = _pad_elem(F3)  # pooling-table row elems

    nc = bacc.Bacc("TRN2", target_bir_lowering=False, debug=False, num_devices=NC,
                   dynamic_dma_scratch_size=int(os.environ.get("KGCN_RING", "16384")))

    # ---- I/O ----
    T1_in = nc.dram_tensor("T1_in", [NC * N_PAD, _pad_elem(XD)], F16,
                           kind="ExternalInput")
    idx_w = nc.dram_tensor("idx_w", [128, E_cap // 16], I16, kind="ExternalInput")
    rel_c = nc.dram_tensor("rel_c", [128, NBLK], F32, kind="ExternalInput")
    sval_c = nc.dram_tensor("sval_c", [128, NBLK], F32, kind="ExternalInput")
    slot_w = nc.dram_tensor("slot_w", [128, NSLOT // 16], I16, kind="ExternalInput")
    dinv_win = nc.dram_tensor("dinv_win", [WIN, NW], F32, kind="ExternalInput")
    W1e = nc.dram_tensor("W1e", [XD + 1, F1], F16, kind="ExternalInput")
    W2e = nc.dram_tensor("W2e", [F1 + 1, F2], F16, kind="ExternalInput")
    W3lo = nc.dram_tensor("W3lo", [128, F3], F16, kind="ExternalInput")
    W3hi = nc.dram_tensor("W3hi", [F2 - 128 + 1, F3], F16, kind="ExternalInput")
    Wg1 = nc.dram_tensor("Wg1", [F3, D1], F16, kind="ExternalInput")
    Wg2 = nc.dram_tensor("Wg2", [D1, D2], F16, kind="ExternalInput")
    Wf1 = nc.dram_tensor("Wf1", [D2, D3], F16, kind="ExternalInput")
    Wf2 = nc.dram_tensor("Wf2", [D3, D4], F16, kind="ExternalInput")
    Wo = nc.dram_tensor("Wo", [D4, 1], F16, kind="ExternalInput")
    bg1 = nc.dram_tensor("bg1", [128, D1 // 128], F32, kind="ExternalInput")
    bg2 = nc.dram_tensor("bg2", [128, D2 // 128], F32, kind="ExternalInput")
    bf1 = nc.dram_tensor("bf1", [128, D3 // 128], F32, kind="ExternalInput")
    bf2 = nc.dram_tensor("bf2", [128, D4 // 128], F32, kind="ExternalInput")
    bo = nc.dram_tensor("bo", [1, 1], F32, kind="ExternalInput")
    WIN2, NW2 = meta["WIN2"], meta["NW2"]
    K2, NBLK2, E2_cap = meta["K2"], meta["NBLK2"], meta["E2_cap"]
    blk_off2 = meta["blk_off2"]
    idx2_w = nc.dram_tensor("idx2_w", [128, E2_cap // 16], I16,
                            kind="ExternalInput")
    rel2_c = nc.dram_tensor("rel2_c", [128, NBLK2], F32, kind="ExternalInput")
    sval2_c = nc.dram_tensor("sval2_c", [128, NBLK2], F32,
                             kind="ExternalInput")
    dinv_c = nc.dram_tensor("dinv_c", [128, N_PAD // 128], F32,
                            kind="ExternalInput")
    dinvsq_win = nc.dram_tensor("dinvsq_win", [WIN, NW], F32,
                                kind="ExternalInput")
    dinvsq_c = nc.dram_tensor("dinvsq_c", [128, N_PAD // 128], F32,
                              kind="ExternalInput")
    iota_in = nc.dram_tensor("iota", [128, WIN2], F16, kind="ExternalInput")
    out_d = nc.dram_tensor("out_d", [1, GPC], F32, kind="ExternalOutput")
    STOP = os.environ.get("KGCN_STOP", "")
    dbg = None
    if STOP:
        dbg = nc.dram_tensor("dbg", [N_PAD, 512], F16, kind="ExternalOutput")

    def _dump(tc, nc, src_dram, rows, cols):
        # copy DRAM region into dbg via SBUF bounce, then stop building
        with tc.tile_pool(name="dbgp", bufs=2) as dp:
            for t in range(_ceil(rows, 128)):
                r0, r1 = t * 128, min((t + 1) * 128, rows)
                dt_ = dp.tile([128, cols], F16, tag="dbg")
                nc.sync.dma_start(dt_[: r1 - r0, :], src_dram[r0:r1, :cols])
                nc.sync.dma_start(dbg[r0:r1, :cols], dt_[: r1 - r0, :])

    REL = mybir.ActivationFunctionType.Relu
    CPY = mybir.ActivationFunctionType.Copy
    SIG = mybir.ActivationFunctionType.Sigmoid
    EQ, MUL, ADD, MAX = (
        mybir.AluOpType.is_equal, mybir.AluOpType.mult,
        mybir.AluOpType.add, mybir.AluOpType.max,
    )

    with tile.TileContext(nc) as tc:
        with (
            tc.tile_pool(name="dramp", bufs=1, space="DRAM") as dramp,
            tc.tile_pool(name="const", bufs=1) as constp,
            tc.tile_pool(name="stream", bufs=1) as streamp,
        ):
            # DRAM scratch: per-core local node tables between layers
            table2 = dramp.tile([N_PAD, E2], F16)
            table3 = dramp.tile([N_PAD, E3], F16)
            h3t = dramp.tile([N_PAD + 1, EP], F16)
            T1 = T1_in

            # persistent SBUF
            idx_sb = streamp.tile([128, E_cap // 16], I16)
            rel_sb = streamp.tile([128, NBLK], F32)
            sval_sb = streamp.tile([128, NBLK], F32)
            slot_sb = streamp.tile([128, NSLOT // 16], I16)
            idx2_sb = streamp.tile([128, E2_cap // 16], I16)
            rel2_sb = streamp.tile([128, NBLK2], F32)
            sval2_sb = streamp.tile([128, NBLK2], F32)
            nc.sync.dma_start(idx_sb[:], idx_w[:, :])
            nc.sync.dma_start(rel_sb[:], rel_c[:, :])
            nc.sync.dma_start(sval_sb[:], sval_c[:, :])
            nc.sync.dma_start(slot_sb[:], slot_w[:, :])
            nc.sync.dma_start(idx2_sb[:], idx2_w[:, :])
            nc.sync.dma_start(rel2_sb[:], rel2_c[:, :])
            nc.sync.dma_start(sval2_sb[:], sval2_c[:, :])

            iota_sb = constp.tile([128, WIN2], F16)
            nc.sync.dma_start(iota_sb[:], iota_in[:, :])
            dwin_sb = constp.tile([WIN, NW], F32)
            nc.sync.dma_start(dwin_sb[:], dinv_win[:, :])
            dinvc_sb = constp.tile([128, N_PAD // 128], F32)
            nc.sync.dma_start(dinvc_sb[:], dinv_c[:, :])
            dsqwin_sb = constp.tile([WIN, NW], F32, name="dsqwin")
            nc.sync.dma_start(dsqwin_sb[:], dinvsq_win[:, :])
            dsqc_sb = constp.tile([128, N_PAD // 128], F32, name="dsqc")
            nc.sync.dma_start(dsqc_sb[:], dinvsq_c[:, :])
            # feature-major dinv^2-scaled self terms (this core's own rows),
            # added to the RS shard post-collective
            selfT2 = streamp.tile([F1, N_PAD], F16, name="selfT2")
            selfT3_lo = streamp.tile([128, N_PAD], F16, name="selfT3lo")
            selfT3_hi = streamp.tile([F2 - 128, N_PAD], F16, name="selfT3hi")
            ident = constp.tile([128, 128], F16)
            make_identity(nc, ident[:])
            zrow = constp.tile([1, EP], F16)
            nc.vector.memset(zrow[:], 0.0)
            nc.sync.dma_start(h3t[0:1, :], zrow[:])


            # ---- conv layers ----
            def conv_layer(li, T_in, ELEM_in, F_in, We_lo, We_hi,
                           F_out, ELEM_out, out_bounce, scale_out,
                           cc_emit=None, out_row0=0, self_dst=None):
                """One GCN layer. We_lo/We_hi: [<=128, F_out] SBUF weight
                tiles covering rows of W [F_in, F_out]; brow: [1, F_out] bias.
                Writes relu(out) (maybe dinv-scaled) into out_bounce rows.
                cc_emit(s): fired right after window-segment s completes
                (dispatches that segment's output AllGather chunk)."""
                GBLK = int(os.environ.get("KGCN_GBLK", "8"))
                NOGATHER = os.environ.get("KGCN_NOGATHER")
                NOSB = os.environ.get("KGCN_NOSB")
                NOMM = os.environ.get("KGCN_NOMM")
                NOEPI = os.environ.get("KGCN_NOEPI")
                nlo = min(F_in, 128)
                nhi = F_in - nlo  # 0 for L1/L2; 28 for L3
                with (
                    tc.tile_pool(name=f"gb{li}", bufs=6) as gbp,
                    tc.tile_pool(name=f"sp{li}", bufs=4) as sp,
                    tc.tile_pool(name=f"slab{li}", bufs=1) as slabp,
                    tc.tile_pool(name=f"ep{li}", bufs=8) as ep,
                    tc.tile_pool(
                        name=f"aps{li}", bufs=(2 if nhi else 4), space="PSUM"
                    ) as aps,
                    tc.tile_pool(name=f"tps{li}", bufs=2, space="PSUM") as tps,
                ):
                    # aggregation slab; row nlo (or nhi in the hi slab) stays
                    # 1.0 so the transform's lhsT carries the bias row
                    slab = slabp.tile([128, NW * WIN], F16)
                    nc.vector.memset(slab[:], 1.0)
                    slab_hi = None
                    if nhi:
                        slab_hi = slabp.tile([128, NW * WIN], F16, tag="slabhi")
                        nc.vector.memset(slab_hi[:], 1.0)

                    def do_window(h, w, pblk, blk0, gtiles):
                        """Emit one window: scatter matmuls + epilogue."""
                        kw = int(K[h, w])
                        ps = aps.tile([128, WIN], F32, tag="pslo")
                        ps_hi = None
                        if nhi:
                            ps_hi = aps.tile(
                                [128, WIN], F32, name="pshi", tag="pshi")
                        if NOMM:
                            nc.vector.memset(ps[:nlo, :], 0.0)
                            if nhi:
                                nc.vector.memset(ps_hi[:nhi, :], 0.0)
                        for j in range([0, kw][not NOMM]):
                            b = blk0 + pblk + j  # global block id
                            gt = gtiles[(pblk + j) // GBLK]
                            ch = (pblk + j) % GBLK
                            st = sp.tile([128, WIN], F16, tag="s")
                            if NOSB:
                                nc.vector.memset(st[:], 0.01)
                            else:
                                nc.vector.tensor_scalar(
                                    st[:], iota_sb[:, :WIN],
                                    rel_sb[:, b : b + 1],
                                    sval_sb[:, b : b + 1], EQ, MUL,
                                )
                            nc.tensor.matmul(
                                ps[:nlo, :], gt[:, ch, :nlo], st[:],
                                start=(j == 0), stop=(j == kw - 1),
                            )
                            if nhi:
                                nc.tensor.matmul(
                                    ps_hi[:nhi, :], gt[:, ch, nlo:F_in],
                                    st[:], start=(j == 0), stop=(j == kw - 1),
                                )
                        ws = slice(w * WIN, (w + 1) * WIN)
                        if h == 0:
                            nc.scalar.activation(slab[:nlo, ws], ps[:nlo, :], CPY)
                            if nhi:
                                nc.scalar.activation(
                                    slab_hi[:nhi, ws], ps_hi[:nhi, :], CPY)
                            return
                        # combine A+B in-place in the slab; the transform
                        # reads one extra preset 1.0 row to add the bias
                        nc.vector.tensor_tensor(
                            slab[:nlo, ws], slab[:nlo, ws], ps[:nlo, :], ADD)
                        if nhi:
                            nc.vector.tensor_tensor(
                                slab_hi[:nhi, ws], slab_hi[:nhi, ws],
                                ps_hi[:nhi, :], ADD)
                        tp = tps.tile([WIN, F_out], F32, tag="tp")
                        if not nhi:
                            nc.tensor.matmul(
                                tp[:], slab[: nlo + 1, ws], We_lo[:],
                                start=True, stop=True)
                        else:
                            nc.tensor.matmul(
                                tp[:], slab[:nlo, ws], We_lo[:],
                                start=True, stop=False)
                            nc.tensor.matmul(
                                tp[:], slab_hi[: nhi + 1, ws], We_hi[:],
                                start=False, stop=True)
                        hs = ep.tile([WIN, ELEM_out], F16, tag="hs")
                        if ELEM_out > F_out:
                            nc.vector.memset(hs[:, F_out:ELEM_out], 0.0)
                        # scale>0 commutes with relu: dinv*relu(x)=relu(dinv*x)
                        nc.scalar.activation(
                            hs[:, :F_out], tp[:], REL,
                            scale=(dwin_sb[:, w : w + 1] if scale_out else 1.0))
                        if self_dst is not None:
                            # dinv^2-scaled rows, feature-major, for the
                            # post-RS self-loop add of the next (push) layer
                            hse = ep.tile([WIN, F_out], F16, tag="hself")
                            nc.scalar.activation(
                                hse[:], tp[:], REL,
                                scale=dsqwin_sb[:, w : w + 1])
                            tsp = tps.tile([F_out, WIN], F16, name="selft",
                                           tag="selft")
                            nc.tensor.transpose(tsp[:], hse[:],
                                                ident[:WIN, :WIN])
                            nc.scalar.activation(
                                self_dst[:, w * WIN : (w + 1) * WIN],
                                tsp[:], CPY)
                        nc.sync.dma_start(
                            out_bounce[out_row0 + w * WIN
                                       : out_row0 + (w + 1) * WIN, :], hs[:])

                    CPW = meta["CPW"]
                    blk0 = 0  # global block counter at segment/pass start
                    for s in range(meta["NSEG"]):
                        for h in range(2):
                            tbl = T_in[h * HALF : (h + 1) * HALF, :]
                            w0, w1 = s * CPW, (s + 1) * CPW
                            seg_blocks = int(K[h, w0:w1].sum())
                            ngath = _ceil(seg_blocks, GBLK)
                            gtiles = []
                            w = w0
                            pblk = 0  # blocks consumed by processed windows
                            issued = 0
                            for g in range(ngath + 1):
                                if g < ngath:
                                    nb = min(GBLK, seg_blocks - g * GBLK)
                                    gt = gbp.tile(
                                        [128, GBLK, ELEM_in], F16, tag="gb")
                                    c0 = (blk0 + g * GBLK) * 8  # idx col off
                                    if NOGATHER:
                                        nc.vector.memset(gt[:, :nb, :], 0.25)
                                    else:
                                        nc.gpsimd.dma_gather(
                                            gt[:, :nb, :], tbl,
                                            idx_sb[:, c0 : c0 + nb * 8],
                                            nb * 128, nb * 128, ELEM_in,
                                        )
                                    gtiles.append(gt)
                                    issued += nb
                                # process fully-gathered windows (all remain
                                # when g == ngath)
                                while w < w1 and pblk + int(K[h, w]) <= issued:
                                    if not NOEPI:
                                        do_window(h, w, pblk, blk0, gtiles)
                                    pblk += int(K[h, w])
                                    w += 1
                            blk0 += seg_blocks
                        if cc_emit is not None:
                            cc_emit(s)

            # weights to SBUF (bias fused as the last row; the matching
            # ones-row lives preset in the aggregation slab)
            w1_sb = constp.tile([XD + 1, F1], F16)
            nc.sync.dma_start(w1_sb[:], W1e[:, :])
            w2_sb = constp.tile([F1 + 1, F2], F16)
            nc.sync.dma_start(w2_sb[:], W2e[:, :])
            w3_lo = constp.tile([128, F3], F16)
            nc.sync.dma_start(w3_lo[:], W3lo[:, :])
            w3_hi = constp.tile([F2 - 128 + 1, F3], F16)
            nc.sync.dma_start(w3_hi[:], W3hi[:, :])

            SL = {"p1": 1, "l1nc": 2, "l1": 2, "l2": 3, "l3": 4,
                  "pool": 5}.get(STOP, 99)
            SEGR = meta["SEGR"]

            # ---- push layer (2/3): scatter local-source messages into a
            # feature-major global accumulator, ReduceScatter the partials,
            # then transform the local shard. No AllGather, no half-split.
            def push_layer(li, T_local, ELEM_in, F_in, We_lo, We_hi,
                           F_out, ELEM_out, out_table, scale_out,
                           out_row0=0, post_hook=None,
                           self_add=None, self_build=None):
                GBLK = 8
                nlo = min(F_in, 128)
                nhi = F_in - nlo
                accum = dramp.tile([NC * F_in, N_PAD], F16,
                                   name=f"accum{li}")
                shard_h = nc.dram_tensor(f"shard{li}", [F_in, N_PAD], F16)
                with (
                    tc.tile_pool(name=f"pgb{li}", bufs=6) as gbp,
                    tc.tile_pool(name=f"psp{li}", bufs=6) as sp,
                    tc.tile_pool(name=f"pslab{li}", bufs=2) as slabp,
                    tc.tile_pool(name=f"paps{li}", bufs=(3 if nhi else 4),
                                 space="PSUM") as aps,
                ):
                    for c in range(NC):
                        slab = slabp.tile([128, N_PAD], F16, tag="sl")
                        slab_hi = None
                        if nhi:
                            slab_hi = slabp.tile([nhi, N_PAD], F16,
                                                 name="slhi", tag="slhi")
                        blk0 = int(blk_off2[c, 0])
                        cblocks = int(K2[c].sum())
                        ngath = _ceil(cblocks, GBLK)
                        gtiles = []
                        w = 0
                        pblk = 0
                        issued = 0
                        for g in range(ngath + 1):
                            if g < ngath:
                                nb = min(GBLK, cblocks - g * GBLK)
                                gt = gbp.tile([128, GBLK, ELEM_in], F16,
                                              tag="gb")
                                c0 = (blk0 + g * GBLK) * 8
                                nc.gpsimd.dma_gather(
                                    gt[:, :nb, :], T_local[:, :],
                                    idx2_sb[:, c0 : c0 + nb * 8],
                                    nb * 128, nb * 128, ELEM_in,
                                )
                                gtiles.append(gt)
                                issued += nb
                            while w < NW2 and pblk + int(K2[c, w]) <= issued:
                                kw = int(K2[c, w])
                                ps = aps.tile([128, WIN2], F32, tag="pps")
                                ps_hi = None
                                if nhi:
                                    ps_hi = aps.tile([128, WIN2], F32,
                                                     name="ppshi", tag="ppshi")
                                for j in range(kw):
                                    b = blk0 + pblk + j
                                    gt = gtiles[(pblk + j) // GBLK]
                                    ch = (pblk + j) % GBLK
                                    st = sp.tile([128, WIN2], F16, tag="ps")
                                    nc.vector.tensor_scalar(
                                        st[:], iota_sb[:],
                                        rel2_sb[:, b : b + 1],
                                        sval2_sb[:, b : b + 1], EQ, MUL,
                                    )
                                    nc.tensor.matmul(
                                        ps[:nlo, :], gt[:, ch, :nlo], st[:],
                                        start=(j == 0), stop=(j == kw - 1),
                                    )
                                    if nhi:
                                        nc.tensor.matmul(
                                            ps_hi[:nhi, :],
                                            gt[:, ch, nlo:F_in], st[:],
                                            start=(j == 0), stop=(j == kw - 1),
                                        )
                                ws = slice(w * WIN2, (w + 1) * WIN2)
                                nc.scalar.activation(
                                    slab[:nlo, ws], ps[:nlo, :], CPY)
                                if nhi:
                                    nc.scalar.activation(
                                        slab_hi[:nhi, ws], ps_hi[:nhi, :], CPY)
                                pblk += kw
                                w += 1
                        nc.sync.dma_start(
                            accum[c * F_in : c * F_in + nlo, :], slab[:nlo, :])
                        if nhi:
                            nc.sync.dma_start(
                                accum[c * F_in + nlo : (c + 1) * F_in, :],
                                slab_hi[:, :])
                nc.gpsimd.collective_compute(
                    "ReduceScatter", mybir.AluOpType.add,
                    replica_groups=[list(range(NC))],
                    ins=[accum[:, :].opt()], outs=[shard_h[:].opt()],
                )
                # transform: relu((dinv·)(aggT·W + b)) per 128-node chunk
                with (
                    tc.tile_pool(name=f"tsh{li}", bufs=1) as shp,
                    tc.tile_pool(name=f"tep{li}", bufs=6) as ep,
                    tc.tile_pool(name=f"ttp{li}", bufs=4, space="PSUM") as tps,
                ):
                    # memset-1.0 full tile first (bias ones row), then DMA
                    # the shard over rows [:nlo] — engine ops can't address
                    # partition ranges off the 32-alignment grid, DMAs can
                    sh_lo = shp.tile([nlo + (0 if nhi else 1), N_PAD], F16)
                    sh_hi = None
                    nc.vector.memset(sh_lo[:, :], 1.0)
                    nc.sync.dma_start(sh_lo[:nlo, :], shard_h[0:nlo, :])
                    if nhi:
                        sh_hi = shp.tile([nhi + 1, N_PAD], F16, name="shhi")
                        nc.vector.memset(sh_hi[:, :], 1.0)
                        nc.sync.dma_start(sh_hi[:nhi, :], shard_h[nlo:F_in, :])
                    if self_add is not None:
                        sa_lo, sa_hi = self_add
                        nc.vector.tensor_tensor(
                            sh_lo[:nlo, :], sh_lo[:nlo, :], sa_lo[:nlo, :],
                            ADD)
                        if sa_hi is not None:
                            nc.vector.tensor_tensor(
                                sh_hi[:nhi, :], sh_hi[:nhi, :],
                                sa_hi[:nhi, :], ADD)
                    for ch in range(N_PAD // 128):
                        cs = slice(ch * 128, (ch + 1) * 128)
                        tp = tps.tile([128, F_out], F32, tag="tp")
                        if not nhi:
                            nc.tensor.matmul(tp[:], sh_lo[:, cs], We_lo[:],
                                             start=True, stop=True)
                        else:
                            nc.tensor.matmul(tp[:], sh_lo[:nlo, cs], We_lo[:],
                                             start=True, stop=False)
                            nc.tensor.matmul(tp[:], sh_hi[:, cs], We_hi[:],
                                             start=False, stop=True)
                        hs = ep.tile([128, ELEM_out], F16, tag="hs")
                        if ELEM_out > F_out:
                            nc.vector.memset(hs[:, F_out:ELEM_out], 0.0)
                        nc.scalar.activation(
                            hs[:, :F_out], tp[:], REL,
                            scale=(dinvc_sb[:, ch : ch + 1]
                                   if scale_out else 1.0))
                        if self_build is not None:
                            sb_lo, sb_hi = self_build
                            hse = ep.tile([128, F_out], F16, tag="hself")
                            nc.scalar.activation(
                                hse[:], tp[:], REL,
                                scale=dsqc_sb[:, ch : ch + 1])
                            for j0, dstt in ((0, sb_lo), (128, sb_hi)):
                                if j0 >= F_out or dstt is None:
                                    continue
                                csz = min(128, F_out - j0)
                                tsp = tps.tile([128, 128], F16, name="selft",
                                               tag="selft")
                                nc.tensor.transpose(
                                    tsp[:csz, :], hse[:, j0 : j0 + csz],
                                    ident[:])
                                nc.scalar.activation(
                                    dstt[:csz, cs], tsp[:csz, :], CPY)
                        nc.sync.dma_start(
                            out_table[out_row0 + ch * 128
                                      : out_row0 + (ch + 1) * 128, :], hs[:])
                        if post_hook is not None:
                            post_hook(ch)

            if SL >= 2:
                conv_layer(1, T1, E1, XD, w1_sb, None, F1, E2, table2,
                           True, cc_emit=None, self_dst=selfT2)
            if SL >= 3:
                push_layer(2, table2, E2, F1, w2_sb, None, F2, E3, table3,
                           True, self_add=(selfT2, None),
                           self_build=(selfT3_lo, selfT3_hi))
            # ---- pooling (two phases): gather h3 rows in slot order,
            # transpose, reduce. Phase A (graphs fully inside the first
            # window segment) is emitted from conv3's segment-0 hook so it
            # overlaps segment 1.
            NCH3 = _ceil(F3, 128)  # feature chunks (3 for 312)
            gtp_cm = tc.tile_pool(name="gtp", bufs=1)
            gtp = gtp_cm.__enter__()
            gT = gtp.tile([128, NCH3 * GPC], F16)
            pgp_cm = tc.tile_pool(name="poolg", bufs=2)
            pgp = pgp_cm.__enter__()
            pps_cm = tc.tile_pool(name="poolps", bufs=2, space="PSUM")
            pps = pps_cm.__enter__()
            PG = int(os.environ.get("KGCN_GBLK", "8"))
            gpg = 128 // SLOT  # graphs per 128-slot tile
            JSTAR = meta["JSTAR"]

            def pool_phase(t0, t1, src_rows):
                # pool slot-tiles [t0, t1) reading h3t rows [0, src_rows)
                ptiles = {}
                for tg in range(t0, t1, PG):
                    nb = min(PG, t1 - tg)
                    pt = pgp.tile([128, PG, EP], F16, name="pg", tag="pg")
                    c0 = tg * 8
                    nc.gpsimd.dma_gather(
                        pt[:, :nb, :], h3t[:src_rows, :],
                        slot_sb[:, c0 : c0 + nb * 8],
                        nb * 128, nb * 128, EP,
                    )
                    ptiles[tg] = pt
                for t in range(t0, t1):
                    pt = ptiles[t0 + ((t - t0) // PG) * PG]
                    ch = (t - t0) % PG
                    for j in range(NCH3):
                        csz = min(128, F3 - j * 128)
                        tps_t = pps.tile(
                            [128, 128], F16, name="tpose", tag="tpose")
                        nc.tensor.transpose(
                            tps_t[:csz, :], pt[:, ch, j * 128 : j * 128 + csz],
                            ident[:])
                        gcol = t * gpg
                        nc.vector.tensor_reduce(
                            gT[:csz, j * GPC + gcol : j * GPC + gcol + gpg],
                            tps_t[:csz, :].rearrange(
                                "p (g s) -> p g s", s=SLOT),
                            mybir.AxisListType.X, MAX,
                        )

            PH = 0 if os.environ.get("KGCN_NOPH") else 1
            HOOK_CH = meta["SEGR"] // 128 - 1
            pool_done = [0]

            def pool_hook(ch):
                # h3t rows [1, SEGR+1) complete once transform chunk HOOK_CH
                # is written; overlap pool phase A with the remaining chunks
                if PH and ch == HOOK_CH and SL >= 5 and JSTAR > 0:
                    pool_phase(0, JSTAR * SLOT // 128, meta["SEGR"] + 1)
                    pool_done[0] = JSTAR * SLOT // 128

            if SL >= 4:
                push_layer(3, table3, E3, F2, w3_lo, w3_hi, F3, EP, h3t,
                           False, out_row0=1, post_hook=pool_hook,
                           self_add=(selfT3_lo, selfT3_hi))
            if SL >= 5:
                pool_phase(pool_done[0], NSLOT // 128, N_PAD + 1)

            # ---- MLP (feature-major; biases per-partition) ----
            if SL >= 6:
              with (
                tc.tile_pool(name="mlpw", bufs=1) as mwp,
                tc.tile_pool(name="mlps", bufs=1) as msp,
                tc.tile_pool(name="mlpps", bufs=4, space="PSUM") as mps,
            ):
                k3 = _ksplits(F3)
                wg1_sb = [mwp.tile([min(128, F3 - k0), D1], F16, name=f"wg1_{i}",
                                   tag=f"wg1_{i}")
                          for i, (k0, k1) in enumerate(k3)]
                for i, (k0, k1) in enumerate(k3):
                    nc.sync.dma_start(wg1_sb[i][:], Wg1[k0:k1, :])
                bg1_sb = msp.tile([128, D1 // 128], F32)
                nc.sync.dma_start(bg1_sb[:], bg1[:, :])
                g1 = msp.tile([128, (D1 // 128) * GPC], F16)
                for m in range(D1 // 128):
                    ps = mps.tile([128, GPC], F32, tag="mlp")
                    for i, (k0, k1) in enumerate(k3):
                        nc.tensor.matmul(
                            ps[:], wg1_sb[i][:, m * 128 : (m + 1) * 128],
                            gT[: k1 - k0, i * GPC : (i + 1) * GPC],
                            start=(i == 0), stop=(i == len(k3) - 1),
                        )
                    nc.scalar.activation(
                        g1[:, m * GPC : (m + 1) * GPC], ps[:], REL,
                        bias=bg1_sb[:, m : m + 1])

                wg2_sb = [mwp.tile([128, D2], F16, name=f"wg2_{i}", tag=f"wg2_{i}")
                          for i in range(D1 // 128)]
                for i in range(D1 // 128):
                    nc.sync.dma_start(wg2_sb[i][:], Wg2[i * 128 : (i + 1) * 128, :])
                bg2_sb = msp.tile([128, D2 // 128], F32)
                nc.sync.dma_start(bg2_sb[:], bg2[:, :])
                g2 = msp.tile([128, GPC], F16)
                ps = mps.tile([128, GPC], F32, tag="mlp")
                for i in range(D1 // 128):
                    nc.tensor.matmul(
                        ps[:], wg2_sb[i][:], g1[:, i * GPC : (i + 1) * GPC],
                        start=(i == 0), stop=(i == D1 // 128 - 1))
                nc.vector.tensor_scalar(g2[:], ps[:], bg2_sb[:, 0:1], None, ADD)

                wf1_sb = mwp.tile([128, D3], F16)
                nc.sync.dma_start(wf1_sb[:], Wf1[:, :])
                bf1_sb = msp.tile([128, D3 // 128], F32)
                nc.sync.dma_start(bf1_sb[:], bf1[:, :])
                c1 = msp.tile([128, (D3 // 128) * GPC], F16)
                for m in range(D3 // 128):
                    ps = mps.tile([128, GPC], F32, tag="mlp")
                    nc.tensor.matmul(
                        ps[:], wf1_sb[:, m * 128 : (m + 1) * 128], g2[:],
                        start=True, stop=True)
                    nc.scalar.activation(
                        c1[:, m * GPC : (m + 1) * GPC], ps[:], REL,
                        bias=bf1_sb[:, m : m + 1])

                wf2_sb = [mwp.tile([128, D4], F16, name=f"wf2_{i}", tag=f"wf2_{i}")
                          for i in range(D3 // 128)]
                for i in range(D3 // 128):
                    nc.sync.dma_start(wf2_sb[i][:], Wf2[i * 128 : (i + 1) * 128, :])
                bf2_sb = msp.tile([128, D4 // 128], F32)
                nc.sync.dma_start(bf2_sb[:], bf2[:, :])
                c2 = msp.tile([128, (D4 // 128) * GPC], F16)
                for m in range(D4 // 128):
                    ps = mps.tile([128, GPC], F32, tag="mlp")
                    for i in range(D3 // 128):
                        nc.tensor.matmul(
                            ps[:], wf2_sb[i][:, m * 128 : (m + 1) * 128],
                            c1[:, i * GPC : (i + 1) * GPC],
                            start=(i == 0), stop=(i == D3 // 128 - 1))
                    nc.scalar.activation(
                        c2[:, m * GPC : (m + 1) * GPC], ps[:], REL,
                        bias=bf2_sb[:, m : m + 1])

                wo_sb = [mwp.tile([128, 1], F16, name=f"wo_{i}", tag=f"wo_{i}")
                         for i in range(D4 // 128)]
                for i in range(D4 // 128):
                    nc.sync.dma_start(wo_sb[i][:], Wo[i * 128 : (i + 1) * 128, :])
                bo_sb = msp.tile([1, 1], F32)
                nc.sync.dma_start(bo_sb[:], bo[:, :])
                pso = mps.tile([1, GPC], F32, tag="mlpo", bufs=1)
                for i in range(D4 // 128):
                    nc.tensor.matmul(
                        pso[:], wo_sb[i][:], c2[:, i * GPC : (i + 1) * GPC],
                        start=(i == 0), stop=(i == D4 // 128 - 1))
                o_sb = msp.tile([1, GPC], F32)
                nc.scalar.activation(o_sb[:], pso[:], SIG, bias=bo_sb[:, 0:1])
                nc.sync.dma_start(out_d[:, :], o_sb[:])
            if STOP == "p1":
                _dump(tc, nc, T1, N_PAD, E1)
            elif STOP in ("l1", "l1nc"):
                _dump(tc, nc, table2, N_PAD, E2)
            elif STOP == "l2":
                _dump(tc, nc, table3, N_PAD, E3)
            elif STOP == "l3":
                _dump(tc, nc, h3t, N_PAD, EP)
            elif STOP == "pool":
                nc.sync.dma_start(dbg[0:128, : NCH3 * GPC], gT[:])
            pps_cm.__exit__(None, None, None)
            pgp_cm.__exit__(None, None, None)
            gtp_cm.__exit__(None, None, None)

    nc.compile()
    return nc


LAST_EXEC_NS = None
LAST_RES = None


def kernel(**inputs):
    global LAST_EXEC_NS, LAST_RES
    x = np.asarray(inputs["x"])
    edge_index = np.asarray(inputs["edge_index"])
    batch = np.asarray(inputs["batch"])
    weights = {k: np.asarray(v) for k, v in inputs.items()
               if k not in ("x", "edge_index", "batch")}
    meta, in_maps = _plan(x, edge_index, batch, weights)
    nc = _build(meta)
    trace = bool(os.environ.get("KGCN_TRACE"))
    res = run_bass_kernel_spmd(nc, in_maps, core_ids=list(range(NC)),
                               trace=trace)
    LAST_RES = res
    LAST_EXEC_NS = res.exec_time_ns
    out = np.concatenate(
        [res.results[c]["out_d"][0] for c in range(NC)]).astype(np.float32)
    return out.reshape(-1, 1)



# revision 44
# speedup vs baseline: 1.0003x; 1.0003x over previous
"""GCN (3x GCNConv + global max pool + MLP) on 8 Trainium2 NeuronCores.

Strategy (graph/data parallel, per sharding hint), all compute in fp16
(fp32 PSUM accumulation):
  - 2048 graphs are split 256-per-core; each core owns the (contiguous)
    nodes of its graphs.
  - GCN algebra is reordered as out = dinv*(A @ (dinv*x)) @ W + b
    (aggregate before transform: F_in-wide messages, not F_out-wide).
  - Layer 1 is PULL: x is replicated, so edges group by local dst window;
    per-edge source rows come via dma_gather (int16 idx, global table in
    two halves), scatter-add is a TensorE matmul against a one-hot "S"
    built on DVE: S[e, m] = (iota[m] == dst_rel[e]) * dinv_dst[e]. The
    transform runs per-window; output = local node table (no collective).
  - Layers 2/3 are PUSH + ReduceScatter: edges group by (dst core, 256-row
    dst window); messages gather from the LOCAL table, scatter-matmul into
    a feature-major accumulator [NC*F_in, N_PAD], one fp16 RS sums the
    partials onto the owning core (RS output = 1/8 of an AllGather, far
    cheaper), then a post-RS transform produces the next local table.
    L2 and L3 share one edge stream (same src idx / dst windows / svals).
  - Self-loops are NOT streamed (they would inflate every window's
    max-over-cores block padding): each layer's transform also emits its
    rows dinv^2-scaled, feature-major (PE transpose); the next layer adds
    that tile to its own RS shard — a core-local, SPMD-uniform add.
  - Global max pool: per-graph fixed 64-slot gather of h3 rows + TensorE
    transpose + DVE reduce_max, phase A overlapped with the L3 transform;
    MLP runs feature-major with per-partition biases; sigmoid on ACT.
"""
import os
import sys

sys.path.insert(0, "/opt/trn_rl_repo")

import numpy as np

import concourse.bass as bass
import concourse.mybir as mybir
import concourse.tile as tile
from concourse import bacc
from concourse.bass_utils import run_bass_kernel_spmd
from concourse.masks import make_identity

F32 = mybir.dt.float32
F16 = mybir.dt.float16
I16 = mybir.dt.int16
NC = 8
WIN = 64  # dst-window size (nodes per scatter matmul group)
NPF16 = np.float16


def _ceil(a, b):
    return -(-a // b)


def _pad_elem(f):
    # gather row length (fp16 elems): must make row bytes a multiple of 256
    return _ceil(f, 128) * 128


def _ksplits(k):
    # split contraction dim k into <=128 chunks
    out = []
    o = 0
    while o < k:
        out.append((o, min(o + 128, k)))
        o += 128
    return out


def _wrap_idx(flat):
    # dma_gather index layout: idx i -> partition i%16, col i//16, replicated x8
    w = flat.reshape(-1, 16).T.astype(np.int16)
    return np.tile(w, (8, 1))


def _plan(x, edge_index, batch, weights):
    N, XD = x.shape
    E = edge_index.shape[1]
    G = 2048 if N == 50000 else int(batch.max()) + 1
    assert G % NC == 0
    GPC = G // NC

    batch = np.asarray(batch, dtype=np.int64)
    sizes = np.bincount(batch, minlength=G)
    gcore = np.arange(G) // GPC
    node_core = gcore[batch]

    core_start = np.searchsorted(batch, np.arange(NC) * GPC)
    core_start = np.concatenate([core_start, [N]])
    ncounts = np.diff(core_start)
    N_PAD = max(128, _ceil(ncounts.max(), 128) * 128)
    assert NC * N_PAD < 65536, "node table too large for int16 half-split"
    HALF = NC * N_PAD // 2
    NW = N_PAD // WIN

    local_row = np.arange(N) - core_start[node_core]
    NCH_CC = 1  # single full-table AllGather (best modeled bandwidth)
    CR = N_PAD // NCH_CC
    NSEG = 2  # window segments (decoupled from collectives; pooling overlap)
    SEGR = N_PAD // NSEG
    assert N_PAD % NCH_CC == 0 and CR % WIN == 0 and SEGR % WIN == 0
    g_row = ((local_row // CR) * (NC * CR) + node_core * CR + local_row % CR)

    src = np.asarray(edge_index[0], dtype=np.int64)
    dst = np.asarray(edge_index[1], dtype=np.int64)
    deg = (np.bincount(dst, minlength=N) + 1).astype(np.float32)
    dinv = (1.0 / np.sqrt(deg)).astype(np.float32)

    src_all = np.concatenate([src, np.arange(N)])
    dst_all = np.concatenate([dst, np.arange(N)])
    e_core = node_core[dst_all]
    e_loc = local_row[dst_all]
    e_w = e_loc // WIN
    e_rel = (e_loc % WIN).astype(np.float32)
    e_grow = g_row[src_all]
    e_half = (e_grow >= HALF).astype(np.int64)
    e_idx16 = (e_grow - e_half * HALF).astype(np.int16)
    e_sval = dinv[dst_all]

    key = (e_core * 2 + e_half) * NW + e_w
    cnt = np.bincount(key, minlength=NC * 2 * NW).reshape(NC, 2, NW)
    K = np.maximum(1, _ceil(cnt.max(axis=0), 128))  # [2, NW] blocks per (half, w)
    NBLK_A, NBLK_B = int(K[0].sum()), int(K[1].sum())
    NBLK = NBLK_A + NBLK_B
    E_cap = NBLK * 128

    # block start offset of each (half, w) group; stream is ordered
    # [seg0: h0 w0..w_cpw-1, h1 w0..][seg1: h0 ...] so each collective
    # chunk's windows finish early in the layer
    CPW = SEGR // WIN
    blk_off = np.zeros((2, NW), dtype=np.int64)
    off = 0
    for s in range(NSEG):
        for h in range(2):
            for w in range(s * CPW, (s + 1) * CPW):
                blk_off[h, w] = off
                off += int(K[h, w])
    assert off == NBLK

    order = np.lexsort((e_w, e_half, e_core))
    s_key = key[order]
    s_idx16 = e_idx16[order]
    s_rel = e_rel[order]
    s_sval = e_sval[order]
    grp_starts = np.searchsorted(s_key, np.arange(NC * 2 * NW))
    grp_ends = np.concatenate([grp_starts[1:], [E + N]])

    idx_w, rel_cols, sval_cols = [], [], []
    for c in range(NC):
        idx16_s = np.zeros(E_cap, np.int16)
        rel_s = np.full(E_cap, -1.0, np.float32)
        sval_s = np.zeros(E_cap, np.float32)
        for h in range(2):
            for w in range(NW):
                kk = (c * 2 + h) * NW + w
                a, b = grp_starts[kk], grp_ends[kk]
                m = b - a
                if m == 0:
                    continue
                d0 = blk_off[h, w] * 128
                idx16_s[d0 : d0 + m] = s_idx16[a:b]
                rel_s[d0 : d0 + m] = s_rel[a:b]
                sval_s[d0 : d0 + m] = s_sval[a:b]
        idx_w.append(_wrap_idx(idx16_s))
        rel_cols.append(np.ascontiguousarray(rel_s.reshape(NBLK, 128).T))
        sval_cols.append(np.ascontiguousarray(sval_s.reshape(NBLK, 128).T))

    # ---- push-mode stream for layers 2/3 (shared): edges grouped by
    # (dst core, 128-wide dst window); messages gathered from the LOCAL
    # node table (src side), scatter-matmul'd into a feature-major global
    # accumulator, then ReduceScatter sums partials onto the dst core.
    WIN2 = 256
    NW2 = N_PAD // WIN2
    # self-loops are NOT streamed: each core adds its own dinv^2-scaled
    # rows to its shard post-ReduceScatter (keeps the max-over-cores block
    # padding flat: the diagonal (own-core) groups would otherwise carry
    # all self-loops and inflate every window's K)
    p_src_core = node_core[src]
    p_loc_src = local_row[src]
    p_dst_core = node_core[dst]
    p_loc_dst = local_row[dst]
    p_w = p_loc_dst // WIN2
    p_rel = (p_loc_dst % WIN2).astype(np.float32)
    p_sval = dinv[dst]
    key2 = (p_src_core * NC + p_dst_core) * NW2 + p_w
    cnt2 = np.bincount(key2, minlength=NC * NC * NW2).reshape(NC, NC, NW2)
    K2 = np.maximum(1, _ceil(cnt2.max(axis=0), 128))  # [NC, NW2]
    NBLK2 = int(K2.sum())
    E2_cap = NBLK2 * 128
    blk_off2 = np.concatenate([[0], np.cumsum(K2.reshape(-1))])[:-1].reshape(
        NC, NW2)

    order2 = np.argsort(key2, kind="stable")
    s2_key = key2[order2]
    s2_idx = p_loc_src[order2].astype(np.int16)
    s2_rel = p_rel[order2]
    s2_sval = p_sval[order2]
    idx2_w, rel2_cols, sval2_cols = [], [], []
    for e in range(NC):
        g0 = np.searchsorted(s2_key, (e * NC) * NW2)
        g1 = np.searchsorted(s2_key, ((e + 1) * NC) * NW2)
        idx16_s = np.zeros(E2_cap, np.int16)
        rel_s = np.full(E2_cap, -1.0, np.float32)
        sval_s = np.zeros(E2_cap, np.float32)
        kk = s2_key[g0:g1] - (e * NC) * NW2  # (c, w) flat group id
        grp_starts2 = np.searchsorted(kk, np.arange(NC * NW2))
        grp_ends2 = np.concatenate([grp_starts2[1:], [g1 - g0]])
        flat_off = blk_off2.reshape(-1)
        for g in range(NC * NW2):
            a, b = grp_starts2[g], grp_ends2[g]
            if a == b:
                continue
            d0 = int(flat_off[g]) * 128
            idx16_s[d0 : d0 + b - a] = s2_idx[g0 + a : g0 + b]
            rel_s[d0 : d0 + b - a] = s2_rel[g0 + a : g0 + b]
            sval_s[d0 : d0 + b - a] = s2_sval[g0 + a : g0 + b]
        idx2_w.append(_wrap_idx(idx16_s))
        rel2_cols.append(np.ascontiguousarray(rel_s.reshape(NBLK2, 128).T))
        sval2_cols.append(np.ascontiguousarray(sval_s.reshape(NBLK2, 128).T))

    # pooling slot plan
    maxg = int(sizes.max())
    SLOT = 64 if maxg <= 64 else 128
    assert maxg <= SLOT, f"graph with {maxg} nodes exceeds slot budget"
    NSLOT = GPC * SLOT
    assert NSLOT % 128 == 0
    # h3 table row 0 is an all-zero dummy; node local row r lives at row r+1
    gstart = np.concatenate([[0], np.cumsum(sizes)])
    slot_w = []
    jstar = GPC
    for c in range(NC):
        sl = np.zeros(NSLOT, np.int64)
        for j in range(GPC):
            gi = c * GPC + j
            st = gstart[gi] - core_start[c]
            sz = sizes[gi]
            sl[j * SLOT : j * SLOT + sz] = np.arange(st + 1, st + sz + 1)
        slot_w.append(_wrap_idx(sl.astype(np.int16)))
        ge = gstart[c * GPC + 1 : (c + 1) * GPC + 1] - core_start[c]
        jstar = min(jstar, int(np.searchsorted(ge, SEGR, side="right")))
    jstar -= jstar % max(1, 128 // SLOT)  # slot-tile granularity
    # graphs complete per 128-row transform chunk (min over cores), in
    # slot-tile units: lets pooling chase the L3 transform chunk-by-chunk
    jchunk = np.full(N_PAD // 128, GPC, np.int64)
    for c in range(NC):
        ge = gstart[c * GPC + 1 : (c + 1) * GPC + 1] - core_start[c]
        for ch in range(N_PAD // 128):
            j = int(np.searchsorted(ge, (ch + 1) * 128, side="right"))
            jchunk[ch] = min(jchunk[ch], j)
    gpt = max(1, 128 // SLOT)  # graphs per slot-tile
    jchunk -= jchunk % gpt

    # per-core node-local aux data; T1 (= dinv*x, padded, chunk-major
    # global layout) is built on the host and uploaded replicated
    dinv_win = []
    dinv_c128 = []
    dinvsq_win = []
    dinvsq_c128 = []
    E1 = _pad_elem(XD)
    T1_host = np.zeros((NC * N_PAD, E1), NPF16)
    xs = (dinv[:, None] * np.asarray(x, np.float32)).astype(NPF16)
    T1_host[g_row, :XD] = xs
    for c in range(NC):
        n0, n1 = core_start[c], core_start[c + 1]
        dl = np.zeros(N_PAD, np.float32)
        dl[: n1 - n0] = dinv[n0:n1]
        dinv_win.append(np.ascontiguousarray(dl.reshape(NW, WIN).T))
        dinv_c128.append(np.ascontiguousarray(dl.reshape(-1, 128).T))
        dinvsq_win.append(np.ascontiguousarray((dl * dl).reshape(NW, WIN).T))
        dinvsq_c128.append(np.ascontiguousarray((dl * dl).reshape(-1, 128).T))

    W1, b1, W2, b2, W3, b3 = (
        weights["W1"], weights["b1"], weights["W2"], weights["b2"],
        weights["W3"], weights["b3"],
    )
    F1, F2, F3 = W1.shape[1], W2.shape[1], W3.shape[1]
    w_shared = dict(
        W1e=np.vstack([W1, b1[None, :]]).astype(NPF16),
        W2e=np.vstack([W2, b2[None, :]]).astype(NPF16),
        W3lo=np.ascontiguousarray(np.asarray(W3[:128], NPF16)),
        W3hi=np.vstack([W3[128:], b3[None, :]]).astype(NPF16),
        Wg1=np.asarray(weights["Wg1"], NPF16),
        Wg2=np.asarray(weights["Wg2"], NPF16),
        Wf1=np.asarray(weights["Wf1"], NPF16),
        Wf2=np.asarray(weights["Wf2"], NPF16),
        Wo=np.asarray(weights["Wo"], NPF16),
        bg1=np.ascontiguousarray(
            np.asarray(weights["bg1"], np.float32).reshape(-1, 128).T),
        bg2=np.asarray(weights["bg2"], np.float32).reshape(128, -1),
        bf1=np.ascontiguousarray(
            np.asarray(weights["bf1"], np.float32).reshape(-1, 128).T),
        bf2=np.ascontiguousarray(
            np.asarray(weights["bf2"], np.float32).reshape(-1, 128).T),
        bo=np.asarray(weights["bo"], np.float32).reshape(1, 1),
        iota=np.broadcast_to(
            np.arange(WIN2, dtype=NPF16)[None, :], (128, WIN2)).copy(),
    )

    meta = dict(
        N=N, XD=XD, G=G, GPC=GPC, N_PAD=N_PAD, HALF=HALF, NW=NW,
        K=K, NBLK_A=NBLK_A, NBLK=NBLK, E_cap=E_cap, NCH_CC=NCH_CC, CR=CR,
        CPW=CPW, NSEG=NSEG, SEGR=SEGR,
        WIN2=WIN2, NW2=NW2, K2=K2, NBLK2=NBLK2, E2_cap=E2_cap,
        blk_off2=blk_off2,
        SLOT=SLOT, NSLOT=NSLOT, JSTAR=jstar, JCHUNK=jchunk,
        F1=F1, F2=F2, F3=F3,
        D1=weights["Wg1"].shape[1], D2=weights["Wg2"].shape[1],
        D3=weights["Wf1"].shape[1], D4=weights["Wf2"].shape[1],
    )
    in_maps = [
        dict(
            T1_in=T1_host, idx_w=idx_w[c], rel_c=rel_cols[c],
            sval_c=sval_cols[c], slot_w=slot_w[c],
            idx2_w=idx2_w[c], rel2_c=rel2_cols[c], sval2_c=sval2_cols[c],
            dinv_win=dinv_win[c], dinv_c=dinv_c128[c],
            dinvsq_win=dinvsq_win[c], dinvsq_c=dinvsq_c128[c],
            **w_shared,
        )
        for c in range(NC)
    ]
    return meta, in_maps


def _build(meta):
    N_PAD, HALF, NW = meta["N_PAD"], meta["HALF"], meta["NW"]
    NCH_CC, CR = meta["NCH_CC"], meta["CR"]
    K, NBLK, E_cap = meta["K"], meta["NBLK"], meta["E_cap"]
    GPC, SLOT, NSLOT = meta["GPC"], meta["SLOT"], meta["NSLOT"]
    F1, F2, F3, XD = meta["F1"], meta["F2"], meta["F3"], meta["XD"]
    D1, D2, D3, D4 = meta["D1"], meta["D2"], meta["D3"], meta["D4"]
    E1, E2, E3 = _pad_elem(XD), _pad_elem(F1), _pad_elem(F2)
    EP = _pad_elem(F3)  # pooling-table row elems

    nc = bacc.Bacc("TRN2", target_bir_lowering=False, debug=False, num_devices=NC,
                   dynamic_dma_scratch_size=int(os.environ.get("KGCN_RING", "16384")))

    # ---- I/O ----
    T1_in = nc.dram_tensor("T1_in", [NC * N_PAD, _pad_elem(XD)], F16,
                           kind="ExternalInput")
    idx_w = nc.dram_tensor("idx_w", [128, E_cap // 16], I16, kind="ExternalInput")
    rel_c = nc.dram_tensor("rel_c", [128, NBLK], F32, kind="ExternalInput")
    sval_c = nc.dram_tensor("sval_c", [128, NBLK], F32, kind="ExternalInput")
    slot_w = nc.dram_tensor("slot_w", [128, NSLOT // 16], I16, kind="ExternalInput")
    dinv_win = nc.dram_tensor("dinv_win", [WIN, NW], F32, kind="ExternalInput")
    W1e = nc.dram_tensor("W1e", [XD + 1, F1], F16, kind="ExternalInput")
    W2e = nc.dram_tensor("W2e", [F1 + 1, F2], F16, kind="ExternalInput")
    W3lo = nc.dram_tensor("W3lo", [128, F3], F16, kind="ExternalInput")
    W3hi = nc.dram_tensor("W3hi", [F2 - 128 + 1, F3], F16, kind="ExternalInput")
    Wg1 = nc.dram_tensor("Wg1", [F3, D1], F16, kind="ExternalInput")
    Wg2 = nc.dram_tensor("Wg2", [D1, D2], F16, kind="ExternalInput")
    Wf1 = nc.dram_tensor("Wf1", [D2, D3], F16, kind="ExternalInput")
    Wf2 = nc.dram_tensor("Wf2", [D3, D4], F16, kind="ExternalInput")
    Wo = nc.dram_tensor("Wo", [D4, 1], F16, kind="ExternalInput")
    bg1 = nc.dram_tensor("bg1", [128, D1 // 128], F32, kind="ExternalInput")
    bg2 = nc.dram_tensor("bg2", [128, D2 // 128], F32, kind="ExternalInput")
    bf1 = nc.dram_tensor("bf1", [128, D3 // 128], F32, kind="ExternalInput")
    bf2 = nc.dram_tensor("bf2", [128, D4 // 128], F32, kind="ExternalInput")
    bo = nc.dram_tensor("bo", [1, 1], F32, kind="ExternalInput")
    WIN2, NW2 = meta["WIN2"], meta["NW2"]
    K2, NBLK2, E2_cap = meta["K2"], meta["NBLK2"], meta["E2_cap"]
    blk_off2 = meta["blk_off2"]
    idx2_w = nc.dram_tensor("idx2_w", [128, E2_cap // 16], I16,
                            kind="ExternalInput")
    rel2_c = nc.dram_tensor("rel2_c", [128, NBLK2], F32, kind="ExternalInput")
    sval2_c = nc.dram_tensor("sval2_c", [128, NBLK2], F32,
                             kind="ExternalInput")
    dinv_c = nc.dram_tensor("dinv_c", [128, N_PAD // 128], F32,
                            kind="ExternalInput")
    dinvsq_win = nc.dram_tensor("dinvsq_win", [WIN, NW], F32,
                                kind="ExternalInput")
    dinvsq_c = nc.dram_tensor("dinvsq_c", [128, N_PAD // 128], F32,
                              kind="ExternalInput")
    iota_in = nc.dram_tensor("iota", [128, WIN2], F16, kind="ExternalInput")
    out_d = nc.dram_tensor("out_d", [1, GPC], F32, kind="ExternalOutput")
    STOP = os.environ.get("KGCN_STOP", "")
    dbg = None
    if STOP:
        dbg = nc.dram_tensor("dbg", [N_PAD, 512], F16, kind="ExternalOutput")

    def _dump(tc, nc, src_dram, rows, cols):
        # copy DRAM region into dbg via SBUF bounce, then stop building
        with tc.tile_pool(name="dbgp", bufs=2) as dp:
            for t in range(_ceil(rows, 128)):
                r0, r1 = t * 128, min((t + 1) * 128, rows)
                dt_ = dp.tile([128, cols], F16, tag="dbg")
                nc.sync.dma_start(dt_[: r1 - r0, :], src_dram[r0:r1, :cols])
                nc.sync.dma_start(dbg[r0:r1, :cols], dt_[: r1 - r0, :])

    REL = mybir.ActivationFunctionType.Relu
    CPY = mybir.ActivationFunctionType.Copy
    SIG = mybir.ActivationFunctionType.Sigmoid
    EQ, MUL, ADD, MAX = (
        mybir.AluOpType.is_equal, mybir.AluOpType.mult,
        mybir.AluOpType.add, mybir.AluOpType.max,
    )

    with tile.TileContext(nc) as tc:
        with (
            tc.tile_pool(name="dramp", bufs=1, space="DRAM") as dramp,
            tc.tile_pool(name="const", bufs=1) as constp,
            tc.tile_pool(name="stream", bufs=1) as streamp,
        ):
            # DRAM scratch: per-core local node tables between layers
            table2 = dramp.tile([N_PAD, E2], F16)
            table3 = dramp.tile([N_PAD, E3], F16)
            h3t = dramp.tile([N_PAD + 1, EP], F16)
            T1 = T1_in

            # persistent SBUF
            idx_sb = streamp.tile([128, E_cap // 16], I16)
            rel_sb = streamp.tile([128, NBLK], F32)
            sval_sb = streamp.tile([128, NBLK], F32)
            slot_sb = streamp.tile([128, NSLOT // 16], I16)
            idx2_sb = streamp.tile([128, E2_cap // 16], I16)
            rel2_sb = streamp.tile([128, NBLK2], F32)
            sval2_sb = streamp.tile([128, NBLK2], F32)
            nc.sync.dma_start(idx_sb[:], idx_w[:, :])
            nc.sync.dma_start(rel_sb[:], rel_c[:, :])
            nc.sync.dma_start(sval_sb[:], sval_c[:, :])
            nc.sync.dma_start(slot_sb[:], slot_w[:, :])
            nc.sync.dma_start(idx2_sb[:], idx2_w[:, :])
            nc.sync.dma_start(rel2_sb[:], rel2_c[:, :])
            nc.sync.dma_start(sval2_sb[:], sval2_c[:, :])

            iota_sb = constp.tile([128, WIN2], F16)
            nc.sync.dma_start(iota_sb[:], iota_in[:, :])
            dwin_sb = constp.tile([WIN, NW], F32)
            nc.sync.dma_start(dwin_sb[:], dinv_win[:, :])
            dinvc_sb = constp.tile([128, N_PAD // 128], F32)
            nc.sync.dma_start(dinvc_sb[:], dinv_c[:, :])
            dsqwin_sb = constp.tile([WIN, NW], F32, name="dsqwin")
            nc.sync.dma_start(dsqwin_sb[:], dinvsq_win[:, :])
            dsqc_sb = constp.tile([128, N_PAD // 128], F32, name="dsqc")
            nc.sync.dma_start(dsqc_sb[:], dinvsq_c[:, :])
            # feature-major dinv^2-scaled self terms (this core's own rows),
            # added to the RS shard post-collective
            selfT2 = streamp.tile([F1, N_PAD], F16, name="selfT2")
            selfT3_lo = streamp.tile([128, N_PAD], F16, name="selfT3lo")
            selfT3_hi = streamp.tile([F2 - 128, N_PAD], F16, name="selfT3hi")
            ident = constp.tile([128, 128], F16)
            make_identity(nc, ident[:])
            zrow = constp.tile([1, EP], F16)
            nc.vector.memset(zrow[:], 0.0)
            nc.sync.dma_start(h3t[0:1, :], zrow[:])


            # ---- conv layers ----
            def conv_layer(li, T_in, ELEM_in, F_in, We_lo, We_hi,
                           F_out, ELEM_out, out_bounce, scale_out,
                           cc_emit=None, out_row0=0, self_dst=None):
                """One GCN layer. We_lo/We_hi: [<=128, F_out] SBUF weight
                tiles covering rows of W [F_in, F_out]; brow: [1, F_out] bias.
                Writes relu(out) (maybe dinv-scaled) into out_bounce rows.
                cc_emit(s): fired right after window-segment s completes
                (dispatches that segment's output AllGather chunk)."""
                GBLK = int(os.environ.get("KGCN_GBLK", "8"))
                NOGATHER = os.environ.get("KGCN_NOGATHER")
                NOSB = os.environ.get("KGCN_NOSB")
                NOMM = os.environ.get("KGCN_NOMM")
                NOEPI = os.environ.get("KGCN_NOEPI")
                nlo = min(F_in, 128)
                nhi = F_in - nlo  # 0 for L1/L2; 28 for L3
                with (
                    tc.tile_pool(name=f"gb{li}", bufs=6) as gbp,
                    tc.tile_pool(name=f"sp{li}", bufs=4) as sp,
                    tc.tile_pool(name=f"slab{li}", bufs=1) as slabp,
                    tc.tile_pool(name=f"ep{li}", bufs=8) as ep,
                    tc.tile_pool(
                        name=f"aps{li}", bufs=(2 if nhi else 4), space="PSUM"
                    ) as aps,
                    tc.tile_pool(name=f"tps{li}", bufs=2, space="PSUM") as tps,
                ):
                    # aggregation slab; row nlo (or nhi in the hi slab) stays
                    # 1.0 so the transform's lhsT carries the bias row
                    slab = slabp.tile([128, NW * WIN], F16)
                    nc.vector.memset(slab[:], 1.0)
                    slab_hi = None
                    if nhi:
                        slab_hi = slabp.tile([128, NW * WIN], F16, tag="slabhi")
                        nc.vector.memset(slab_hi[:], 1.0)

                    def do_window(h, w, pblk, blk0, gtiles):
                        """Emit one window: scatter matmuls + epilogue."""
                        kw = int(K[h, w])
                        ps = aps.tile([128, WIN], F32, tag="pslo")
                        ps_hi = None
                        if nhi:
                            ps_hi = aps.tile(
                                [128, WIN], F32, name="pshi", tag="pshi")
                        if NOMM:
                            nc.vector.memset(ps[:nlo, :], 0.0)
                            if nhi:
                                nc.vector.memset(ps_hi[:nhi, :], 0.0)
                        for j in range([0, kw][not NOMM]):
                            b = blk0 + pblk + j  # global block id
                            gt = gtiles[(pblk + j) // GBLK]
                            ch = (pblk + j) % GBLK
                            st = sp.tile([128, WIN], F16, tag="s")
                            if NOSB:
                                nc.vector.memset(st[:], 0.01)
                            else:
                                nc.vector.tensor_scalar(
                                    st[:], iota_sb[:, :WIN],
                                    rel_sb[:, b : b + 1],
                                    sval_sb[:, b : b + 1], EQ, MUL,
                                )
                            nc.tensor.matmul(
                                ps[:nlo, :], gt[:, ch, :nlo], st[:],
                                start=(j == 0), stop=(j == kw - 1),
                            )
                            if nhi:
                                nc.tensor.matmul(
                                    ps_hi[:nhi, :], gt[:, ch, nlo:F_in],
                                    st[:], start=(j == 0), stop=(j == kw - 1),
                                )
                        ws = slice(w * WIN, (w + 1) * WIN)
                        if h == 0:
                            nc.scalar.activation(slab[:nlo, ws], ps[:nlo, :], CPY)
                            if nhi:
                                nc.scalar.activation(
                                    slab_hi[:nhi, ws], ps_hi[:nhi, :], CPY)
                            return
                        # combine A+B in-place in the slab; the transform
                        # reads one extra preset 1.0 row to add the bias
                        nc.vector.tensor_tensor(
                            slab[:nlo, ws], slab[:nlo, ws], ps[:nlo, :], ADD)
                        if nhi:
                            nc.vector.tensor_tensor(
                                slab_hi[:nhi, ws], slab_hi[:nhi, ws],
                                ps_hi[:nhi, :], ADD)
                        tp = tps.tile([WIN, F_out], F32, tag="tp")
                        if not nhi:
                            nc.tensor.matmul(
                                tp[:], slab[: nlo + 1, ws], We_lo[:],
                                start=True, stop=True)
                        else:
                            nc.tensor.matmul(
                                tp[:], slab[:nlo, ws], We_lo[:],
                                start=True, stop=False)
                            nc.tensor.matmul(
                                tp[:], slab_hi[: nhi + 1, ws], We_hi[:],
                                start=False, stop=True)
                        hs = ep.tile([WIN, ELEM_out], F16, tag="hs")
                        if ELEM_out > F_out:
                            nc.vector.memset(hs[:, F_out:ELEM_out], 0.0)
                        # scale>0 commutes with relu: dinv*relu(x)=relu(dinv*x)
                        nc.scalar.activation(
                            hs[:, :F_out], tp[:], REL,
                            scale=(dwin_sb[:, w : w + 1] if scale_out else 1.0))
                        if self_dst is not None:
                            # dinv^2-scaled rows, feature-major, for the
                            # post-RS self-loop add of the next (push) layer
                            hse = ep.tile([WIN, F_out], F16, tag="hself")
                            nc.scalar.activation(
                                hse[:], tp[:], REL,
                                scale=dsqwin_sb[:, w : w + 1])
                            tsp = tps.tile([F_out, WIN], F16, name="selft",
                                           tag="selft")
                            nc.tensor.transpose(tsp[:], hse[:],
                                                ident[:WIN, :WIN])
                            nc.scalar.activation(
                                self_dst[:, w * WIN : (w + 1) * WIN],
                                tsp[:], CPY)
                        nc.sync.dma_start(
                            out_bounce[out_row0 + w * WIN
                                       : out_row0 + (w + 1) * WIN, :], hs[:])

                    CPW = meta["CPW"]
                    blk0 = 0  # global block counter at segment/pass start
                    for s in range(meta["NSEG"]):
                        for h in range(2):
                            tbl = T_in[h * HALF : (h + 1) * HALF, :]
                            w0, w1 = s * CPW, (s + 1) * CPW
                            seg_blocks = int(K[h, w0:w1].sum())
                            ngath = _ceil(seg_blocks, GBLK)
                            gtiles = []
                            w = w0
                            pblk = 0  # blocks consumed by processed windows
                            issued = 0
                            for g in range(ngath + 1):
                                if g < ngath:
                                    nb = min(GBLK, seg_blocks - g * GBLK)
                                    gt = gbp.tile(
                                        [128, GBLK, ELEM_in], F16, tag="gb")
                                    c0 = (blk0 + g * GBLK) * 8  # idx col off
                                    if NOGATHER:
                                        nc.vector.memset(gt[:, :nb, :], 0.25)
                                    else:
                                        nc.gpsimd.dma_gather(
                                            gt[:, :nb, :], tbl,
                                            idx_sb[:, c0 : c0 + nb * 8],
                                            nb * 128, nb * 128, ELEM_in,
                                        )
                                    gtiles.append(gt)
                                    issued += nb
                                # process fully-gathered windows (all remain
                                # when g == ngath)
                                while w < w1 and pblk + int(K[h, w]) <= issued:
                                    if not NOEPI:
                                        do_window(h, w, pblk, blk0, gtiles)
                                    pblk += int(K[h, w])
                                    w += 1
                            blk0 += seg_blocks
                        if cc_emit is not None:
                            cc_emit(s)

            # weights to SBUF (bias fused as the last row; the matching
            # ones-row lives preset in the aggregation slab)
            w1_sb = constp.tile([XD + 1, F1], F16)
            nc.sync.dma_start(w1_sb[:], W1e[:, :])
            w2_sb = constp.tile([F1 + 1, F2], F16)
            nc.sync.dma_start(w2_sb[:], W2e[:, :])
            w3_lo = constp.tile([128, F3], F16)
            nc.sync.dma_start(w3_lo[:], W3lo[:, :])
            w3_hi = constp.tile([F2 - 128 + 1, F3], F16)
            nc.sync.dma_start(w3_hi[:], W3hi[:, :])

            SL = {"p1": 1, "l1nc": 2, "l1": 2, "l2": 3, "l3": 4,
                  "pool": 5}.get(STOP, 99)
            SEGR = meta["SEGR"]

            # ---- push layer (2/3): scatter local-source messages into a
            # feature-major global accumulator, ReduceScatter the partials,
            # then transform the local shard. No AllGather, no half-split.
            def push_layer(li, T_local, ELEM_in, F_in, We_lo, We_hi,
                           F_out, ELEM_out, out_table, scale_out,
                           out_row0=0, post_hook=None,
                           self_add=None, self_build=None):
                GBLK = 8
                nlo = min(F_in, 128)
                nhi = F_in - nlo
                accum = dramp.tile([NC * F_in, N_PAD], F16,
                                   name=f"accum{li}")
                shard_h = nc.dram_tensor(f"shard{li}", [F_in, N_PAD], F16)
                with (
                    tc.tile_pool(name=f"pgb{li}", bufs=6) as gbp,
                    tc.tile_pool(name=f"psp{li}", bufs=4) as sp,
                    tc.tile_pool(name=f"pslab{li}", bufs=2) as slabp,
                    tc.tile_pool(name=f"paps{li}", bufs=(2 if nhi else 4),
                                 space="PSUM") as aps,
                ):
                    for c in range(NC):
                        slab = slabp.tile([128, N_PAD], F16, tag="sl")
                        slab_hi = None
                        if nhi:
                            slab_hi = slabp.tile([nhi, N_PAD], F16,
                                                 name="slhi", tag="slhi")
                        blk0 = int(blk_off2[c, 0])
                        cblocks = int(K2[c].sum())
                        ngath = _ceil(cblocks, GBLK)
                        gtiles = []
                        w = 0
                        pblk = 0
                        issued = 0
                        for g in range(ngath + 1):
                            if g < ngath:
                                nb = min(GBLK, cblocks - g * GBLK)
                                gt = gbp.tile([128, GBLK, ELEM_in], F16,
                                              tag="gb")
                                c0 = (blk0 + g * GBLK) * 8
                                nc.gpsimd.dma_gather(
                                    gt[:, :nb, :], T_local[:, :],
                                    idx2_sb[:, c0 : c0 + nb * 8],
                                    nb * 128, nb * 128, ELEM_in,
                                )
                                gtiles.append(gt)
                                issued += nb
                            while w < NW2 and pblk + int(K2[c, w]) <= issued:
                                kw = int(K2[c, w])
                                ps = aps.tile([128, WIN2], F32, tag="pps")
                                ps_hi = None
                                if nhi:
                                    ps_hi = aps.tile([128, WIN2], F32,
                                                     name="ppshi", tag="ppshi")
                                for j in range(kw):
                                    b = blk0 + pblk + j
                                    gt = gtiles[(pblk + j) // GBLK]
                                    ch = (pblk + j) % GBLK
                                    st = sp.tile([128, WIN2], F16, tag="ps")
                                    nc.vector.tensor_scalar(
                                        st[:], iota_sb[:],
                                        rel2_sb[:, b : b + 1],
                                        sval2_sb[:, b : b + 1], EQ, MUL,
                                    )
                                    nc.tensor.matmul(
                                        ps[:nlo, :], gt[:, ch, :nlo], st[:],
                                        start=(j == 0), stop=(j == kw - 1),
                                    )
                                    if nhi:
                                        nc.tensor.matmul(
                                            ps_hi[:nhi, :],
                                            gt[:, ch, nlo:F_in], st[:],
                                            start=(j == 0), stop=(j == kw - 1),
                                        )
                                ws = slice(w * WIN2, (w + 1) * WIN2)
                                nc.scalar.activation(
                                    slab[:nlo, ws], ps[:nlo, :], CPY)
                                if nhi:
                                    nc.scalar.activation(
                                        slab_hi[:nhi, ws], ps_hi[:nhi, :], CPY)
                                pblk += kw
                                w += 1
                        nc.sync.dma_start(
                            accum[c * F_in : c * F_in + nlo, :], slab[:nlo, :])
                        if nhi:
                            nc.sync.dma_start(
                                accum[c * F_in + nlo : (c + 1) * F_in, :],
                                slab_hi[:, :])
                nc.gpsimd.collective_compute(
                    "ReduceScatter", mybir.AluOpType.add,
                    replica_groups=[list(range(NC))],
                    ins=[accum[:, :].opt()], outs=[shard_h[:].opt()],
                )
                # transform: relu((dinv·)(aggT·W + b)) per 128-node chunk
                with (
                    tc.tile_pool(name=f"tsh{li}", bufs=1) as shp,
                    tc.tile_pool(name=f"tep{li}", bufs=6) as ep,
                    tc.tile_pool(name=f"ttp{li}", bufs=4, space="PSUM") as tps,
                ):
                    # memset-1.0 full tile first (bias ones row), then DMA
                    # the shard over rows [:nlo] — engine ops can't address
                    # partition ranges off the 32-alignment grid, DMAs can
                    sh_lo = shp.tile([nlo + (0 if nhi else 1), N_PAD], F16)
                    sh_hi = None
                    nc.vector.memset(sh_lo[:, :], 1.0)
                    nc.sync.dma_start(sh_lo[:nlo, :], shard_h[0:nlo, :])
                    if nhi:
                        sh_hi = shp.tile([nhi + 1, N_PAD], F16, name="shhi")
                        nc.vector.memset(sh_hi[:, :], 1.0)
                        nc.sync.dma_start(sh_hi[:nhi, :], shard_h[nlo:F_in, :])
                    if self_add is not None:
                        sa_lo, sa_hi = self_add
                        nc.vector.tensor_tensor(
                            sh_lo[:nlo, :], sh_lo[:nlo, :], sa_lo[:nlo, :],
                            ADD)
                        if sa_hi is not None:
                            nc.vector.tensor_tensor(
                                sh_hi[:nhi, :], sh_hi[:nhi, :],
                                sa_hi[:nhi, :], ADD)
                    for ch in range(N_PAD // 128):
                        cs = slice(ch * 128, (ch + 1) * 128)
                        tp = tps.tile([128, F_out], F32, tag="tp")
                        if not nhi:
                            nc.tensor.matmul(tp[:], sh_lo[:, cs], We_lo[:],
                                             start=True, stop=True)
                        else:
                            nc.tensor.matmul(tp[:], sh_lo[:nlo, cs], We_lo[:],
                                             start=True, stop=False)
                            nc.tensor.matmul(tp[:], sh_hi[:, cs], We_hi[:],
                                             start=False, stop=True)
                        hs = ep.tile([128, ELEM_out], F16, tag="hs")
                        if ELEM_out > F_out:
                            nc.vector.memset(hs[:, F_out:ELEM_out], 0.0)
                        nc.scalar.activation(
                            hs[:, :F_out], tp[:], REL,
                            scale=(dinvc_sb[:, ch : ch + 1]
                                   if scale_out else 1.0))
                        if self_build is not None:
                            sb_lo, sb_hi = self_build
                            hse = ep.tile([128, F_out], F16, tag="hself")
                            nc.scalar.activation(
                                hse[:], tp[:], REL,
                                scale=dsqc_sb[:, ch : ch + 1])
                            for j0, dstt in ((0, sb_lo), (128, sb_hi)):
                                if j0 >= F_out or dstt is None:
                                    continue
                                csz = min(128, F_out - j0)
                                tsp = tps.tile([128, 128], F16, name="selft",
                                               tag="selft")
                                nc.tensor.transpose(
                                    tsp[:csz, :], hse[:, j0 : j0 + csz],
                                    ident[:])
                                nc.scalar.activation(
                                    dstt[:csz, cs], tsp[:csz, :], CPY)
                        nc.sync.dma_start(
                            out_table[out_row0 + ch * 128
                                      : out_row0 + (ch + 1) * 128, :], hs[:])
                        if post_hook is not None:
                            post_hook(ch)

            if SL >= 2:
                conv_layer(1, T1, E1, XD, w1_sb, None, F1, E2, table2,
                           True, cc_emit=None, self_dst=selfT2)
            if SL >= 3:
                push_layer(2, table2, E2, F1, w2_sb, None, F2, E3, table3,
                           True, self_add=(selfT2, None),
                           self_build=(selfT3_lo, selfT3_hi))
            # ---- pooling (two phases): gather h3 rows in slot order,
            # transpose, reduce. Phase A (graphs fully inside the first
            # window segment) is emitted from conv3's segment-0 hook so it
            # overlaps segment 1.
            NCH3 = _ceil(F3, 128)  # feature chunks (3 for 312)
            gtp_cm = tc.tile_pool(name="gtp", bufs=1)
            gtp = gtp_cm.__enter__()
            gT = gtp.tile([128, NCH3 * GPC], F16)
            pgp_cm = tc.tile_pool(name="poolg", bufs=2)
            pgp = pgp_cm.__enter__()
            pps_cm = tc.tile_pool(name="poolps", bufs=2, space="PSUM")
            pps = pps_cm.__enter__()
            PG = int(os.environ.get("KGCN_GBLK", "8"))
            gpg = 128 // SLOT  # graphs per 128-slot tile
            JSTAR = meta["JSTAR"]

            def pool_phase(t0, t1, src_rows):
                # pool slot-tiles [t0, t1) reading h3t rows [0, src_rows)
                ptiles = {}
                for tg in range(t0, t1, PG):
                    nb = min(PG, t1 - tg)
                    pt = pgp.tile([128, PG, EP], F16, name="pg", tag="pg")
                    c0 = tg * 8
                    nc.gpsimd.dma_gather(
                        pt[:, :nb, :], h3t[:src_rows, :],
                        slot_sb[:, c0 : c0 + nb * 8],
                        nb * 128, nb * 128, EP,
                    )
                    ptiles[tg] = pt
                for t in range(t0, t1):
                    pt = ptiles[t0 + ((t - t0) // PG) * PG]
                    ch = (t - t0) % PG
                    for j in range(NCH3):
                        csz = min(128, F3 - j * 128)
                        tps_t = pps.tile(
                            [128, 128], F16, name="tpose", tag="tpose")
                        nc.tensor.transpose(
                            tps_t[:csz, :], pt[:, ch, j * 128 : j * 128 + csz],
                            ident[:])
                        gcol = t * gpg
                        nc.vector.tensor_reduce(
                            gT[:csz, j * GPC + gcol : j * GPC + gcol + gpg],
                            tps_t[:csz, :].rearrange(
                                "p (g s) -> p g s", s=SLOT),
                            mybir.AxisListType.X, MAX,
                        )

            PH = 0 if os.environ.get("KGCN_NOPH") else 1
            HOOK_CH = meta["SEGR"] // 128 - 1
            pool_done = [0]

            def pool_hook(ch):
                # h3t rows [1, SEGR+1) complete once transform chunk HOOK_CH
                # is written; overlap pool phase A with the remaining chunks
                if PH and ch == HOOK_CH and SL >= 5 and JSTAR > 0:
                    pool_phase(0, JSTAR * SLOT // 128, meta["SEGR"] + 1)
                    pool_done[0] = JSTAR * SLOT // 128

            if SL >= 4:
                push_layer(3, table3, E3, F2, w3_lo, w3_hi, F3, EP, h3t,
                           False, out_row0=1, post_hook=pool_hook,
                           self_add=(selfT3_lo, selfT3_hi))
            if SL >= 5:
                pool_phase(pool_done[0], NSLOT // 128, N_PAD + 1)

            # ---- MLP (feature-major; biases per-partition) ----
            if SL >= 6:
              with (
                tc.tile_pool(name="mlpw", bufs=1) as mwp,
                tc.tile_pool(name="mlps", bufs=1) as msp,
                tc.tile_pool(name="mlpps", bufs=4, space="PSUM") as mps,
            ):
                k3 = _ksplits(F3)
                wg1_sb = [mwp.tile([min(128, F3 - k0), D1], F16, name=f"wg1_{i}",
                                   tag=f"wg1_{i}")
                          for i, (k0, k1) in enumerate(k3)]
                for i, (k0, k1) in enumerate(k3):
                    nc.sync.dma_start(wg1_sb[i][:], Wg1[k0:k1, :])
                bg1_sb = msp.tile([128, D1 // 128], F32)
                nc.sync.dma_start(bg1_sb[:], bg1[:, :])
                g1 = msp.tile([128, (D1 // 128) * GPC], F16)
                for m in range(D1 // 128):
                    ps = mps.tile([128, GPC], F32, tag="mlp")
                    for i, (k0, k1) in enumerate(k3):
                        nc.tensor.matmul(
                            ps[:], wg1_sb[i][:, m * 128 : (m + 1) * 128],
                            gT[: k1 - k0, i * GPC : (i + 1) * GPC],
                            start=(i == 0), stop=(i == len(k3) - 1),
                        )
                    nc.scalar.activation(
                        g1[:, m * GPC : (m + 1) * GPC], ps[:], REL,
                        bias=bg1_sb[:, m : m + 1])

                wg2_sb = [mwp.tile([128, D2], F16, name=f"wg2_{i}", tag=f"wg2_{i}")
                          for i in range(D1 // 128)]
                for i in range(D1 // 128):
                    nc.sync.dma_start(wg2_sb[i][:], Wg2[i * 128 : (i + 1) * 128, :])
                bg2_sb = msp.tile([128, D2 // 128], F32)
                nc.sync.dma_start(bg2_sb[:], bg2[:, :])
                g2 = msp.tile([128, GPC], F16)
                ps = mps.tile([128, GPC], F32, tag="mlp")
                for i in range(D1 // 128):
                    nc.tensor.matmul(
                        ps[:], wg2_sb[i][:], g1[:, i * GPC : (i + 1) * GPC],
                        start=(i == 0), stop=(i == D1 // 128 - 1))
                nc.vector.tensor_scalar(g2[:], ps[:], bg2_sb[:, 0:1], None, ADD)

                wf1_sb = mwp.tile([128, D3], F16)
                nc.sync.dma_start(wf1_sb[:], Wf1[:, :])
                bf1_sb = msp.tile([128, D3 // 128], F32)
                nc.sync.dma_start(bf1_sb[:], bf1[:, :])
                c1 = msp.tile([128, (D3 // 128) * GPC], F16)
                for m in range(D3 // 128):
                    ps = mps.tile([128, GPC], F32, tag="mlp")
                    nc.tensor.matmul(
                        ps[:], wf1_sb[:, m * 128 : (m + 1) * 128], g2[:],
                        start=True, stop=True)
                    nc.scalar.activation(
                        c1[:, m * GPC : (m + 1) * GPC], ps[:], REL,
                        bias=bf1_sb[:, m : m + 1])

                wf2_sb = [mwp.tile([128, D4], F16, name=f"wf2_{i}", tag=f"wf2_{i}")
                          for i in range(D3 // 128)]
                for i in range(D3 // 128):
                    nc.sync.dma_start(wf2_sb[i][:], Wf2[i * 128 : (i + 1) * 128, :])
                bf2_sb = msp.tile([128, D4 // 128], F32)
                nc.sync.dma_start(bf2_sb[:], bf2[:, :])
                c2 = msp.tile([128, (D4 // 128) * GPC], F16)
                for m in range(D4 // 128):
                    ps = mps.tile([128, GPC], F32, tag="mlp")
                    for i in range(D3 // 128):
                        nc.tensor.matmul(
                            ps[:], wf2_sb[i][:, m * 128 : (m + 1) * 128],
                            c1[:, i * GPC : (i + 1) * GPC],
                            start=(i == 0), stop=(i == D3 // 128 - 1))
                    nc.scalar.activation(
                        c2[:, m * GPC : (m + 1) * GPC], ps[:], REL,
                        bias=bf2_sb[:, m : m + 1])

                wo_sb = [mwp.tile([128, 1], F16, name=f"wo_{i}", tag=f"wo_{i}")
                         for i in range(D4 // 128)]
                for i in range(D4 // 128):
                    nc.sync.dma_start(wo_sb[i][:], Wo[i * 128 : (i + 1) * 128, :])
                bo_sb = msp.tile([1, 1], F32)
                nc.sync.dma_start(bo_sb[:], bo[:, :])
                pso = mps.tile([1, GPC], F32, tag="mlpo", bufs=1)
                for i in range(D4 // 128):
                    nc.tensor.matmul(
                        pso[:], wo_sb[i][:], c2[:, i * GPC : (i + 1) * GPC],
                        start=(i == 0), stop=(i == D4 // 128 - 1))
                o_sb = msp.tile([1, GPC], F32)
                nc.scalar.activation(o_sb[:], pso[:], SIG, bias=bo_sb[:, 0:1])
                nc.sync.dma_start(out_d[:, :], o_sb[:])
            if STOP == "p1":
                _dump(tc, nc, T1, N_PAD, E1)
            elif STOP in ("l1", "l1nc"):
                _dump(tc, nc, table2, N_PAD, E2)
            elif STOP == "l2":
                _dump(tc, nc, table3, N_PAD, E3)
            elif STOP == "l3":
                _dump(tc, nc, h3t, N_PAD, EP)
            elif STOP == "pool":
                nc.sync.dma_start(dbg[0:128, : NCH3 * GPC], gT[:])
            pps_cm.__exit__(None, None, None)
            pgp_cm.__exit__(None, None, None)
            gtp_cm.__exit__(None, None, None)

    nc.compile()
    return nc


LAST_EXEC_NS = None
LAST_RES = None


def kernel(**inputs):
    global LAST_EXEC_NS, LAST_RES
    x = np.asarray(inputs["x"])
    edge_index = np.asarray(inputs["edge_index"])
    batch = np.asarray(inputs["batch"])
    weights = {k: np.asarray(v) for k, v in inputs.items()
               if k not in ("x", "edge_index", "batch")}
    meta, in_maps = _plan(x, edge_index, batch, weights)
    nc = _build(meta)
    trace = bool(os.environ.get("KGCN_TRACE"))
    res = run_bass_kernel_spmd(nc, in_maps, core_ids=list(range(NC)),
                               trace=trace)
    LAST_RES = res
    LAST_EXEC_NS = res.exec_time_ns
    out = np.concatenate(
        [res.results[c]["out_d"][0] for c in range(NC)]).astype(np.float32)
    return out.reshape(-1, 1)

